# revision 2
# baseline (speedup 1.0000x reference)
"""Trainium2 Bass kernel for additive (Bahdanau-style) attention.

Reference computation (fp32):
    enc    = encoder_output.transpose(1, 0, 2)            # [B, S, F]
    concat = [enc, broadcast(decoder_hidden)]             # [B, S, F+D]
    h      = tanh(concat @ W1.T + b1)                     # [B, S, D]
    scores = h @ W2.T + b2                                # [B, S, 1]
    alpha  = softmax(scores, axis=S)
    out    = einsum('bs,bsf->bf', alpha[..., 0], enc)[:, None, :]

Sharding: data-parallel over batch — 8 NeuronCores x 4 batches each;
weights replicated.

The dominant hT = W1e^T @ enc matmul runs on the PE in fp8e4m3 DoubleRow
mode (K=256 per instruction, 2x fp16 MACs/cycle): enc and W1e quantized to
e4m3, W1e pre-scaled by 16 to clear the e4m3 denormal floor, the 1/16
riding the tanh activation's scale operand.

W-residual accuracy trick (key to passing the 2e-2 gate at pure-fp8 speed):
h's only consumer is the scalar score w2^T tanh(z), so the effect of the
W-quantization residual dW linearizes to a rank-1 per-position correction
    delta_score[s] ~= c^T enc_s,   c = dW @ (0.55 * w2)
which rides the score PSUM as four extra M=128 DR matmuls per S-block
(dual-fp8 ldweights require exactly 128 stationary columns; cols 1..127
are zero).  w2 is pre-scaled by 1024 so c fits fp8; 1/1024 rides the exp
activation's scale.  End-to-end max-rel-err 1.42e-2 measured on HW.

The softmax weighted sum out = alpha^T enc runs on the DVE from an fp16
copy of enc (scalar_tensor_tensor with free-dim accumulate); softmax uses
a constant exp shift (scores are O(1), max subtraction unnecessary).
"""

import json
import os

import numpy as np
import ml_dtypes

import concourse.bass as bass
import concourse.mybir as mybir
import concourse.tile as tile
from concourse.bass_utils import run_bass_kernel_spmd


def _split_multi_waits(nc: bass.Bass) -> None:
    """Walrus in this container rejects instructions with >1 sync-wait
    command. Split extras into standalone single-wait EventSemaphore
    instructions placed immediately before, on the same engine."""
    d = json.loads(mybir.module_to_json_string(nc.m))
    n_new = 0
    for f in d.get("functions", []):
        for bb in f.get("blocks", []):
            new_insts = []
            for ins in bb.get("instructions", []):
                si = ins.get("sync_info") or {}
                ow = si.get("on_wait") or []
                if len(ow) > 1:
                    for w in ow[:-1]:
                        n_new += 1
                        new_insts.append({
                            "debug": ins.get("debug", 0),
                            "engine": ins["engine"],
                            "ins": [],
                            "outs": [],
                            "name": f"{ins['name']}-swait{n_new}",
                            "opcode": "EventSemaphore",
                            "sync_info": {"on_update": [], "on_wait": [w]},
                        })
                    si["on_wait"] = [ow[-1]]
                new_insts.append(ins)
            bb["instructions"] = new_insts
    if n_new:
        fixed = json.dumps(d).encode()
        nc.to_json_bytes = lambda: fixed  # type: ignore[method-assign]

F16 = mybir.dt.float16
F32 = mybir.dt.float32
F8 = mybir.dt.float8e4
DR = mybir.MatmulPerfMode.DoubleRow
NPF8 = ml_dtypes.float8_e4m3

N_CORES = 8
EXP_SHIFT = -3.0
PE_WS_LAST = False
QUANT = "fp8_wcorr"
HILO_ORDER = "q_major"
EBC = "pe"
STT16 = False
W_SCALE = 16.0
# fp8_wcorr: rank-1 score correction for the W-residual. The correction
# operand c = dW @ (C0 * w2) rides the score psum; w2 is pre-scaled by
# C_SCALE so c fits fp8, and 1/C_SCALE rides the exp activation's scale.
C0 = 0.55
C_SCALE = 1024.0


class Cfg:
    def __init__(self, S=1024, F=1024, D=512, DEC=512, BPC=4):
        self.S, self.F, self.D, self.DEC, self.BPC = S, F, D, DEC, BPC
        self.NFT = F // 128      # fp16 feature tiles (weighted-sum path)
        self.NQ = F // 256       # fp8 DoubleRow K-pair tiles
        self.NM = D // 128
        self.NKD = DEC // 128
        self.SBLK = min(512, S)
        self.NSB = S // self.SBLK
        assert F % 256 == 0 and D % 128 == 0 and DEC % 128 == 0
        assert S % self.SBLK == 0


FULL = Cfg()


def build_bass(cfg: Cfg, repeat: int = 1) -> bass.Bass:
    S, F, D, DEC, BPC = cfg.S, cfg.F, cfg.D, cfg.DEC, cfg.BPC
    NFT, NQ, NM, NKD, SBLK, NSB = (
        cfg.NFT, cfg.NQ, cfg.NM, cfg.NKD, cfg.SBLK, cfg.NSB
    )
    fp8 = QUANT in ("fp8_pure", "fp8_wlo", "fp8_wcorr")
    use_wlo = QUANT == "fp8_wlo"
    use_wcorr = QUANT == "fp8_wcorr"
    assert not (use_wcorr and PE_WS_LAST), "wcorr needs PE_WS=0"
    sc_scale = (1.0 / C_SCALE) if use_wcorr else 1.0

    nc = bass.Bass()

    enc_in = nc.dram_tensor("enc_in", [BPC, NFT, 128, S], F16, kind="ExternalInput")
    if fp8:
        enc8_in = nc.dram_tensor("enc8", [BPC, NQ, 128, 2, S], F8,
                                 kind="ExternalInput")
        w1e8h_in = nc.dram_tensor("w1e8h", [NQ, 128, 2, D], F8,
                                  kind="ExternalInput")
        if use_wlo:
            w1e8l_in = nc.dram_tensor("w1e8l", [NQ, 128, 2, D], F8,
                                      kind="ExternalInput")
        if use_wcorr:
            # M=128 (columns 1..127 zero): dual-fp8 ldweights requires
            # exactly 128 stationary columns
            c8_in = nc.dram_tensor("c8", [NQ, 128, 2, 128], F8,
                                   kind="ExternalInput")
    else:
        w1e_in = nc.dram_tensor("w1e", [NFT, 128, D], F16, kind="ExternalInput")
    w1d_in = nc.dram_tensor("w1d", [NKD, 128, D], F16, kind="ExternalInput")
    b1_in = nc.dram_tensor("b1r", [1, D], F16, kind="ExternalInput")
    if use_wcorr:
        w2_in = nc.dram_tensor("w2t", [NM, 128, 2], F16, kind="ExternalInput")
    else:
        w2_in = nc.dram_tensor("w2t", [NM, 128], F16, kind="ExternalInput")
    dec_in = nc.dram_tensor("dect", [NKD, 128, BPC], F16, kind="ExternalInput")
    out_dram = nc.dram_tensor("out", [BPC, 128, NFT], F32, kind="ExternalOutput")
    if PE_WS_LAST:
        sf_in = nc.dram_tensor("enc_sf", [S // 128, 128, F], F16,
                               kind="ExternalInput")
        out2_dram = nc.dram_tensor("out2", [1, F], F32, kind="ExternalOutput")

    Tanh = mybir.ActivationFunctionType.Tanh
    Exp = mybir.ActivationFunctionType.Exp
    AX = mybir.AxisListType.X
    MUL = mybir.AluOpType.mult
    ADD = mybir.AluOpType.add

    h_scale = (1.0 / W_SCALE) if fp8 else 1.0

    if EBC == "pool":
        from concourse import library_config
        nc.gpsimd.load_library(library_config.attn)

    with tile.TileContext(nc) as tc:
        with (
            tc.tile_pool(name="consts", bufs=1) as consts,
            tc.tile_pool(name="encp", bufs=3 * NFT) as encp,
            tc.tile_pool(name="enc8p", bufs=3 * NQ) as enc8p,
            tc.tile_pool(name="thp", bufs=2 * NM + 2) as thp,
            tc.tile_pool(name="ep", bufs=3) as ep_pool,
            tc.tile_pool(name="zp", bufs=3) as zp,
            tc.tile_pool(name="scrp", bufs=4) as scrp,
            tc.tile_pool(name="sfp", bufs=2) as sfp,
            tc.tile_pool(name="etp", bufs=2) as etp,
            tc.tile_pool(name="accp", bufs=3) as accp,
            tc.tile_pool(name="outp", bufs=3) as outp_pool,
            tc.tile_pool(name="hps", bufs=4, space="PSUM") as hps,
            tc.tile_pool(name="scps", bufs=2, space="PSUM") as scps,
            tc.tile_pool(name="miscps", bufs=2, space="PSUM") as miscps,
        ):
            # ---- constants (small dec-path operands first: PE work sooner) ----
            w1d_sb = consts.tile([128, NKD, D], F16)
            nc.sync.dma_start(out=w1d_sb, in_=w1d_in.rearrange("a p d -> p a d"))
            b1_sb = consts.tile([1, D], F16)
            nc.sync.dma_start(out=b1_sb, in_=b1_in[:, :])
            if use_wcorr:
                w2_sb = consts.tile([128, NM, 2], F16)
                nc.sync.dma_start(out=w2_sb,
                                  in_=w2_in.rearrange("a p o -> p a o"))
            else:
                w2_sb = consts.tile([128, NM], F16)
                nc.sync.dma_start(out=w2_sb, in_=w2_in.rearrange("a p -> p a"))
            dect_sb = consts.tile([128, NKD, BPC], F16)
            nc.sync.dma_start(out=dect_sb, in_=dec_in.rearrange("a p b -> p a b"))

            enc0_8 = []
            enc0_tiles = []
            if fp8:
                # interleave weight chunks with batch-0 fp8 enc tiles so the
                # PE can start the kt-outer first block while streaming
                w1e8h_sb = consts.tile([128, NQ, 2, D], F8)
                if use_wlo:
                    w1e8l_sb = consts.tile([128, NQ, 2, D], F8)
                if use_wcorr:
                    c8_sb = consts.tile([128, NQ, 2, 128], F8)
                    nc.sync.dma_start(
                        out=c8_sb, in_=c8_in.rearrange("a p t o -> p a t o")
                    )
                for q in range(NQ):
                    nc.sync.dma_start(out=w1e8h_sb[:, q], in_=w1e8h_in[q])
                    if use_wlo:
                        nc.sync.dma_start(out=w1e8l_sb[:, q], in_=w1e8l_in[q])
                    e8t = enc8p.tile([128, 2, S], F8, tag="enc8_t", name="enc8_t")
                    nc.sync.dma_start(out=e8t, in_=enc8_in[0, q])
                    enc0_8.append(e8t)
                for ft in range(NFT):
                    et0 = encp.tile([128, S], F16, tag="enc_t", name="enc_t")
                    nc.sync.dma_start(out=et0, in_=enc_in[0, ft])
                    enc0_tiles.append(et0)
            else:
                w1e_sb = consts.tile([128, NFT, D], F16)
                for kt in range(NFT):
                    nc.sync.dma_start(out=w1e_sb[:, kt, :], in_=w1e_in[kt])
                    et0 = encp.tile([128, S], F16, tag="enc_t", name="enc_t")
                    nc.sync.dma_start(out=et0, in_=enc_in[0, kt])
                    enc0_tiles.append(et0)

            onesb = consts.tile([1, BPC], F16)
            nc.vector.memset(onesb, 1.0)
            ones16 = consts.tile([1, 128], F16)
            nc.vector.memset(ones16, 1.0)
            ones32 = consts.tile([1, 128], F32)
            nc.vector.memset(ones32, 1.0)
            shift_sb = consts.tile([1, 1], F32)
            nc.vector.memset(shift_sb, EXP_SHIFT)
            if PE_WS_LAST:
                shift128 = consts.tile([128, 1], F32)
                nc.vector.memset(shift128, EXP_SHIFT)

            # ---- dec_proj^T + b1:  [128 (d%), NM * BPC] fp32 ----
            dec_sb = consts.tile([128, NM, BPC], F32)
            for m in range(NM):
                dp = miscps.tile([128, BPC], F32, tag="mps")
                for kd in range(NKD):
                    nc.tensor.matmul(
                        dp,
                        lhsT=w1d_sb[:, kd, m * 128:(m + 1) * 128],
                        rhs=dect_sb[:, kd, :],
                        start=(kd == 0),
                        stop=False,
                    )
                nc.tensor.matmul(
                    dp,
                    lhsT=b1_sb[0:1, m * 128:(m + 1) * 128],
                    rhs=onesb,
                    start=False,
                    stop=True,
                )
                nc.scalar.copy(dec_sb[:, m, :], dp)

            def h_matmuls(hp, m, e8_tiles, e16_tiles, sl, q_outer_bufs=None):
                """Accumulate one m-tile of hT over the full F contraction."""
                if fp8:
                    w_list = [w1e8h_sb] + ([w1e8l_sb] if use_wlo else [])
                    if HILO_ORDER == "q_major":
                        seq = [(w_sb, q) for q in range(NQ) for w_sb in w_list]
                    else:
                        seq = [(w_sb, q) for w_sb in w_list for q in range(NQ)]
                    for j, (w_sb, q) in enumerate(seq):
                        nc.tensor.matmul(
                            hp,
                            lhsT=w_sb[:, q, :, m * 128:(m + 1) * 128],
                            rhs=e8_tiles[q][:, :, sl],
                            start=(j == 0),
                            stop=(j == len(seq) - 1),
                            perf_mode=DR,
                        )
                else:
                    for kt in range(NFT):
                        nc.tensor.matmul(
                            hp,
                            lhsT=w1e_sb[:, kt, m * 128:(m + 1) * 128],
                            rhs=e16_tiles[kt][:, sl],
                            start=(kt == 0),
                            stop=(kt == NFT - 1),
                        )

            # ---- main loop over local batches ----
            for iter_i, b in enumerate(i % BPC for i in range(repeat * BPC)):
                if iter_i == 0:
                    enc_tiles = enc0_tiles
                    enc8_tiles = enc0_8
                else:
                    enc8_tiles = []
                    if fp8:
                        for q in range(NQ):
                            e8t = enc8p.tile([128, 2, S], F8, tag="enc8_t")
                            nc.sync.dma_start(out=e8t, in_=enc8_in[b, q])
                            enc8_tiles.append(e8t)
                    enc_tiles = []
                    for ft in range(NFT):
                        et = encp.tile([128, S], F16, tag="enc_t")
                        nc.sync.dma_start(out=et, in_=enc_in[b, ft])
                        enc_tiles.append(et)

                blocks = [(i * SBLK, SBLK) for i in range(NSB)]

                pe_ws = PE_WS_LAST and b == BPC - 1
                if pe_ws:
                    sf_sb = sfp.tile([128, S // 128, F], F16, tag="sf")
                    nc.sync.dma_start(
                        out=sf_sb, in_=sf_in.rearrange("a p f -> p a f")
                    )
                    eT_sb = etp.tile([128, S // 128], F16, tag="eT")

                e16 = ep_pool.tile([1, S], F16)
                zacc = zp.tile([1, len(blocks)], F32, tag="zacc")
                acc2 = accp.tile([128, len(blocks), NFT], F32)

                first_block = iter_i == 0
                for sb, (boff, bsz) in enumerate(blocks):
                    sl = slice(boff, boff + bsz)
                    th_tiles = []
                    if first_block and sb == 0 and fp8:
                        # q-outer: every arriving enc8 tile feeds its matmuls
                        hp_list = [hps.tile([128, bsz], F32, tag="hp", name="hp")
                                   for _ in range(NM)]
                        passes = [(w1e8h_sb, True)] + (
                            [(w1e8l_sb, False)] if use_wlo else []
                        )
                        last_w = passes[-1][0]
                        for q in range(NQ):
                            for w_sb, is_first in passes:
                                for m in range(NM):
                                    nc.tensor.matmul(
                                        hp_list[m],
                                        lhsT=w_sb[:, q, :, m * 128:(m + 1) * 128],
                                        rhs=enc8_tiles[q][:, :, sl],
                                        start=(is_first and q == 0),
                                        stop=(w_sb is last_w and q == NQ - 1),
                                        perf_mode=DR,
                                    )
                        for m in range(NM):
                            th = thp.tile([128, bsz], F16, tag="th")
                            nc.scalar.activation(
                                th, hp_list[m], Tanh,
                                bias=dec_sb[:, m, b:b + 1], scale=h_scale,
                            )
                            th_tiles.append(th)
                    elif first_block and sb == 0:
                        hp_list = [hps.tile([128, bsz], F32, tag="hp", name="hp")
                                   for _ in range(NM)]
                        for kt in range(NFT):
                            for m in range(NM):
                                nc.tensor.matmul(
                                    hp_list[m],
                                    lhsT=w1e_sb[:, kt, m * 128:(m + 1) * 128],
                                    rhs=enc_tiles[kt][:, sl],
                                    start=(kt == 0),
                                    stop=(kt == NFT - 1),
                                )
                        for m in range(NM):
                            th = thp.tile([128, bsz], F16, tag="th")
                            nc.scalar.activation(
                                th, hp_list[m], Tanh,
                                bias=dec_sb[:, m, b:b + 1], scale=h_scale,
                            )
                            th_tiles.append(th)
                    else:
                        for m in range(NM):
                            hp = hps.tile([128, bsz], F32, tag="hp")
                            h_matmuls(hp, m, enc8_tiles, enc_tiles, sl)
                            th = thp.tile([128, bsz], F16, tag="th")
                            nc.scalar.activation(
                                th, hp, Tanh, bias=dec_sb[:, m, b:b + 1],
                                scale=h_scale,
                            )
                            th_tiles.append(th)

                    if use_wcorr:
                        # rank-1 W-residual correction rides the score psum
                        # (runs early: only needs enc8, not th). M=128 because
                        # dual-fp8 ldweights requires it; rows 1..127 junk.
                        sc2 = scps.tile([128, bsz], F32, tag="sc")
                        for q in range(NQ):
                            nc.tensor.matmul(
                                sc2,
                                lhsT=c8_sb[:, q, :, :],
                                rhs=enc8_tiles[q][:, :, sl],
                                start=(q == 0),
                                stop=False,
                                perf_mode=DR,
                            )
                        sc = sc2[0:1, :]
                        for m in range(NM):
                            nc.tensor.matmul(
                                sc,
                                lhsT=w2_sb[:, m, 0:1],
                                rhs=th_tiles[m],
                                start=False,
                                stop=(m == NM - 1),
                            )
                    else:
                        sc = scps.tile([1, bsz], F32, tag="sc")
                        for m in range(NM):
                            nc.tensor.matmul(
                                sc,
                                lhsT=w2_sb[:, m:m + 1],
                                rhs=th_tiles[m],
                                start=(m == 0),
                                stop=(m == NM - 1),
                            )
                    # e = exp(scores * sc_scale + EXP_SHIFT); zacc[sb] = sum(e)
                    nc.scalar.activation(
                        e16[0:1, sl], sc, Exp,
                        bias=shift_sb[0:1, 0:1],
                        scale=sc_scale,
                        accum_out=zacc[0:1, sb:sb + 1],
                    )
                    if pe_ws:
                        for c in range(bsz // 128):
                            scT = miscps.tile([128, 1], F32, tag="mps",
                                              name="scT")
                            for kd in range(NM):
                                nc.tensor.matmul(
                                    scT,
                                    lhsT=th_tiles[kd][:, c * 128:(c + 1) * 128],
                                    rhs=w2_sb[:, kd:kd + 1],
                                    start=(kd == 0),
                                    stop=(kd == NM - 1),
                                )
                            ci = boff // 128 + c
                            nc.scalar.activation(
                                eT_sb[:, ci:ci + 1], scT, Exp,
                                bias=shift128[:, 0:1],
                            )
                        continue
                    if EBC == "pool":
                        ebc16 = scrp.tile([128, bsz], F16, tag="ebc16")
                        nc.gpsimd.partition_broadcast(ebc16, e16[0:1, sl])
                        ebc = ebc16
                    else:
                        ebc = miscps.tile([128, bsz], F32, tag="mps")
                        nc.tensor.matmul(
                            ebc, lhsT=ones16, rhs=e16[0:1, sl],
                            start=True, stop=True,
                        )
                        if STT16:
                            ebc16 = scrp.tile([128, bsz], F16, tag="ebc16")
                            nc.vector.tensor_copy(ebc16, ebc)
                            ebc = ebc16
                    for ft in range(NFT):
                        scr = scrp.tile([128, bsz], F16, tag="scr")
                        nc.vector.scalar_tensor_tensor(
                            out=scr,
                            in0=enc_tiles[ft][:, sl],
                            scalar=1.0,
                            in1=ebc,
                            op0=MUL,
                            op1=MUL,
                            accum_out=acc2[:, sb, ft:ft + 1],
                        )

                z = zp.tile([1, 1], F32, tag="z")
                nc.vector.tensor_reduce(z, zacc, axis=AX, op=ADD)
                invz = zp.tile([1, 1], F32, tag="invz")
                nc.vector.reciprocal(invz, z)

                if pe_ws:
                    o1 = outp_pool.tile([1, F], F32, tag="o1")
                    for fo in range(0, F, 512):
                        w = min(512, F - fo)
                        aps = hps.tile([1, w], F32, tag="hp", name="aps")
                        for kt in range(S // 128):
                            nc.tensor.matmul(
                                aps,
                                lhsT=eT_sb[:, kt:kt + 1],
                                rhs=sf_sb[:, kt, fo:fo + w],
                                start=(kt == 0),
                                stop=(kt == S // 128 - 1),
                            )
                        nc.scalar.activation(
                            o1[0:1, fo:fo + w], aps,
                            mybir.ActivationFunctionType.Copy,
                            scale=invz[0:1, 0:1],
                        )
                    nc.sync.dma_start(out=out2_dram[0:1, :], in_=o1)
                    continue

                izp = scps.tile([128, 1], F32, tag="sc")
                nc.tensor.matmul(izp, lhsT=ones32, rhs=invz, start=True, stop=True)
                izb = zp.tile([128, 1], F32, tag="izb")
                nc.scalar.copy(izb, izp)

                o = outp_pool.tile([128, NFT], F32)
                nblk = len(blocks)
                if nblk == 1:
                    nc.vector.tensor_scalar_mul(o, acc2[:, 0, :], izb)
                else:
                    osum = outp_pool.tile([128, NFT], F32, tag="osum")
                    nc.vector.tensor_add(osum, acc2[:, 0, :], acc2[:, 1, :])
                    for sb in range(2, nblk):
                        nc.vector.tensor_add(osum, osum, acc2[:, sb, :])
                    nc.vector.tensor_scalar_mul(o, osum, izb)
                nc.sync.dma_start(out=out_dram[b], in_=o)

    _split_multi_waits(nc)
    return nc


def prep_inputs(encoder_output, decoder_hidden, W1, b1, W2, cfg: Cfg):
    """Host-side sharding + layout. Returns per-core input maps."""
    S, F, D, DEC, BPC = cfg.S, cfg.F, cfg.D, cfg.DEC, cfg.BPC
    NFT, NQ, NM, NKD = cfg.NFT, cfg.NQ, cfg.NM, cfg.NKD
    n_cores = encoder_output.shape[1] // BPC
    fp8 = QUANT in ("fp8_pure", "fp8_wlo", "fp8_wcorr")

    # enc [S, B, F] -> [B, F, S] -> f = p*NFT + ft -> [B, NFT, 128, S] fp16
    enc_t = np.ascontiguousarray(encoder_output.transpose(1, 2, 0))
    enc_l = np.ascontiguousarray(
        enc_t.reshape(-1, 128, NFT, S).transpose(0, 2, 1, 3)
    ).astype(np.float16)

    w1e_t = W1[:, :F].T.astype(np.float32)  # [F, D]
    if fp8:
        # f = q*256 + i*128 + p  (DoubleRow pair layout)
        enc8_l = np.ascontiguousarray(
            enc_t.reshape(-1, NQ, 2, 128, S).transpose(0, 1, 3, 2, 4)
        ).astype(NPF8)
        w16 = w1e_t * W_SCALE
        whi = w16.astype(NPF8)
        wlo = (w16 - whi.astype(np.float32)).astype(NPF8)
        w1e8h = np.ascontiguousarray(
            whi.reshape(NQ, 2, 128, D).transpose(0, 2, 1, 3)
        )
        w1e8l = np.ascontiguousarray(
            wlo.reshape(NQ, 2, 128, D).transpose(0, 2, 1, 3)
        )
        if QUANT == "fp8_wcorr":
            dW = w1e_t - whi.astype(np.float32) / W_SCALE  # [F, D]
            c = dW @ (W2.reshape(-1) * C0)                 # [F]
            c8 = (c * C_SCALE).astype(NPF8)
            c8_l = np.zeros((NQ, 128, 2, 128), dtype=NPF8)
            c8_l[..., 0] = c8.reshape(NQ, 2, 128).transpose(0, 2, 1)
    else:
        w1e = np.ascontiguousarray(
            w1e_t.reshape(128, NFT, D).transpose(1, 0, 2)
        ).astype(np.float16)

    w1d_t = W1[:, F:].T.astype(np.float32)  # [DEC, D]
    w1d = np.ascontiguousarray(w1d_t.reshape(NKD, 128, D)).astype(np.float16)

    b1r = b1.reshape(1, D).astype(np.float16)
    if QUANT == "fp8_wcorr":
        w2t = np.zeros((NM, 128, 2), dtype=np.float16)
        w2t[..., 0] = (W2.reshape(D) * C_SCALE).reshape(NM, 128)
    else:
        w2t = W2.reshape(D).reshape(NM, 128).astype(np.float16)
    dect_full = decoder_hidden.T.reshape(NKD, 128, -1).astype(np.float16)

    in_maps = []
    for c in range(n_cores):
        bs = slice(c * BPC, (c + 1) * BPC)
        m = {
            "enc_in": enc_l[bs],
            "w1d": w1d,
            "b1r": b1r,
            "w2t": w2t,
            "dect": np.ascontiguousarray(dect_full[:, :, bs]),
        }
        if fp8:
            m["enc8"] = enc8_l[bs]
            m["w1e8h"] = w1e8h
            if QUANT == "fp8_wlo":
                m["w1e8l"] = w1e8l
            if QUANT == "fp8_wcorr":
                m["c8"] = c8_l
        else:
            m["w1e"] = w1e
        if PE_WS_LAST:
            sf = np.ascontiguousarray(
                encoder_output[:, c * BPC + BPC - 1, :]
            ).astype(np.float16)
            m["enc_sf"] = sf.reshape(S // 128, 128, F)
        in_maps.append(m)
    return in_maps


def assemble_core(r, cfg: Cfg):
    """Per-core output rows [BPC, F] from the result map."""
    out = r["out"].reshape(cfg.BPC, cfg.F).copy()
    if PE_WS_LAST:
        out[cfg.BPC - 1] = r["out2"][0]
    return out


def kernel(encoder_output, decoder_hidden, W1, b1, W2, b2):
    """Full inputs in, full output out. b2 cancels in the softmax."""
    encoder_output = np.asarray(encoder_output, dtype=np.float32)
    decoder_hidden = np.asarray(decoder_hidden, dtype=np.float32)
    W1 = np.asarray(W1, dtype=np.float32)
    b1 = np.asarray(b1, dtype=np.float32)
    W2 = np.asarray(W2, dtype=np.float32)

    cfg = FULL
    nc = build_bass(cfg)
    in_maps = prep_inputs(encoder_output, decoder_hidden, W1, b1, W2, cfg)
    res = run_bass_kernel_spmd(nc, in_maps, list(range(N_CORES)))
    out = np.concatenate(
        [assemble_core(r, cfg) for r in res.results], axis=0
    )
    return out[:, None, :].astype(np.float32)


if __name__ == "__main__":
    import reference

    inputs = reference.setup_inputs()
    expected = np.asarray(reference.reference(**inputs))
    actual = kernel(**{k: np.asarray(v) for k, v in inputs.items()})
    err = np.abs(actual - expected).max() / np.abs(expected).max()
    print("Relative error:", err)


# revision 3
# speedup vs baseline: 1.0353x; 1.0353x over previous
"""Trainium2 Bass kernel for additive (Bahdanau-style) attention.

Reference computation (fp32):
    enc    = encoder_output.transpose(1, 0, 2)            # [B, S, F]
    concat = [enc, broadcast(decoder_hidden)]             # [B, S, F+D]
    h      = tanh(concat @ W1.T + b1)                     # [B, S, D]
    scores = h @ W2.T + b2                                # [B, S, 1]
    alpha  = softmax(scores, axis=S)
    out    = einsum('bs,bsf->bf', alpha[..., 0], enc)[:, None, :]

Sharding: data-parallel over batch — 8 NeuronCores x 4 batches each;
weights replicated.

The dominant hT = W1e^T @ enc matmul runs on the PE in fp8e4m3 DoubleRow
mode (K=256 per instruction, 2x fp16 MACs/cycle): enc and W1e quantized to
e4m3, W1e pre-scaled by 16 to clear the e4m3 denormal floor, the 1/16
riding the tanh activation's scale operand.

W-residual accuracy trick (key to passing the 2e-2 gate at pure-fp8 speed):
h's only consumer is the scalar score w2^T tanh(z), so the effect of the
W-quantization residual dW linearizes to a rank-1 per-position correction
    delta_score[s] ~= c^T enc_s,   c = dW @ (0.55 * w2)
which rides the score PSUM as four extra M=128 DR matmuls per S-block
(dual-fp8 ldweights require exactly 128 stationary columns; cols 1..127
are zero).  w2 is pre-scaled by 1024 so c fits fp8; 1/1024 rides the exp
activation's scale.  End-to-end max-rel-err 1.42e-2 measured on HW.

The softmax weighted sum out = alpha^T enc runs on the DVE from an fp16
copy of enc (scalar_tensor_tensor with free-dim accumulate); softmax uses
a constant exp shift (scores are O(1), max subtraction unnecessary).
"""

import json
import os

import numpy as np
import ml_dtypes

import concourse.bass as bass
import concourse.mybir as mybir
import concourse.tile as tile
from concourse.bass_utils import run_bass_kernel_spmd


def _split_multi_waits(nc: bass.Bass) -> None:
    """Walrus in this container rejects instructions with >1 sync-wait
    command. Split extras into standalone single-wait EventSemaphore
    instructions placed immediately before, on the same engine."""
    d = json.loads(mybir.module_to_json_string(nc.m))
    n_new = 0
    for f in d.get("functions", []):
        for bb in f.get("blocks", []):
            new_insts = []
            for ins in bb.get("instructions", []):
                si = ins.get("sync_info") or {}
                ow = si.get("on_wait") or []
                if len(ow) > 1:
                    for w in ow[:-1]:
                        n_new += 1
                        new_insts.append({
                            "debug": ins.get("debug", 0),
                            "engine": ins["engine"],
                            "ins": [],
                            "outs": [],
                            "name": f"{ins['name']}-swait{n_new}",
                            "opcode": "EventSemaphore",
                            "sync_info": {"on_update": [], "on_wait": [w]},
                        })
                    si["on_wait"] = [ow[-1]]
                new_insts.append(ins)
            bb["instructions"] = new_insts
    if n_new:
        fixed = json.dumps(d).encode()
        nc.to_json_bytes = lambda: fixed  # type: ignore[method-assign]

F16 = mybir.dt.float16
F32 = mybir.dt.float32
F8 = mybir.dt.float8e4
DR = mybir.MatmulPerfMode.DoubleRow
NPF8 = ml_dtypes.float8_e4m3

N_CORES = 8
EXP_SHIFT = -3.0
PE_WS_LAST = False
QUANT = "fp8_wcorr"
HILO_ORDER = "q_major"
EBC = "pe"
STT16 = False
W_SCALE = 16.0
# fp8_wcorr: rank-1 score correction for the W-residual. The correction
# operand c = dW @ (C0 * w2) rides the score psum; w2 is pre-scaled by
# C_SCALE so c fits fp8, and 1/C_SCALE rides the exp activation's scale.
C0 = 0.55
C_SCALE = 1024.0


class Cfg:
    def __init__(self, S=1024, F=1024, D=512, DEC=512, BPC=4):
        self.S, self.F, self.D, self.DEC, self.BPC = S, F, D, DEC, BPC
        self.NFT = F // 128      # fp16 feature tiles (weighted-sum path)
        self.NQ = F // 256       # fp8 DoubleRow K-pair tiles
        self.NM = D // 128
        self.NKD = DEC // 128
        self.SBLK = min(512, S)
        self.NSB = S // self.SBLK
        assert F % 256 == 0 and D % 128 == 0 and DEC % 128 == 0
        assert S % self.SBLK == 0


FULL = Cfg()


def build_bass(cfg: Cfg, repeat: int = 1) -> bass.Bass:
    S, F, D, DEC, BPC = cfg.S, cfg.F, cfg.D, cfg.DEC, cfg.BPC
    NFT, NQ, NM, NKD, SBLK, NSB = (
        cfg.NFT, cfg.NQ, cfg.NM, cfg.NKD, cfg.SBLK, cfg.NSB
    )
    fp8 = QUANT in ("fp8_pure", "fp8_wlo", "fp8_wcorr")
    use_wlo = QUANT == "fp8_wlo"
    use_wcorr = QUANT == "fp8_wcorr"
    assert not (use_wcorr and PE_WS_LAST), "wcorr needs PE_WS=0"
    sc_scale = (1.0 / C_SCALE) if use_wcorr else 1.0

    nc = bass.Bass()

    enc_in = nc.dram_tensor("enc_in", [BPC, NFT, 128, S], F16, kind="ExternalInput")
    if fp8:
        enc8_in = nc.dram_tensor("enc8", [BPC, NQ, 128, 2, S], F8,
                                 kind="ExternalInput")
        w1e8h_in = nc.dram_tensor("w1e8h", [NQ, 128, 2, D], F8,
                                  kind="ExternalInput")
        if use_wlo:
            w1e8l_in = nc.dram_tensor("w1e8l", [NQ, 128, 2, D], F8,
                                      kind="ExternalInput")
        if use_wcorr:
            # M=128 (columns 1..127 zero): dual-fp8 ldweights requires
            # exactly 128 stationary columns
            c8_in = nc.dram_tensor("c8", [NQ, 128, 2, 128], F8,
                                   kind="ExternalInput")
    else:
        w1e_in = nc.dram_tensor("w1e", [NFT, 128, D], F16, kind="ExternalInput")
    w1d_in = nc.dram_tensor("w1d", [NKD, 128, D], F16, kind="ExternalInput")
    b1_in = nc.dram_tensor("b1r", [1, D], F16, kind="ExternalInput")
    if use_wcorr:
        w2_in = nc.dram_tensor("w2t", [NM, 128, 2], F16, kind="ExternalInput")
    else:
        w2_in = nc.dram_tensor("w2t", [NM, 128], F16, kind="ExternalInput")
    dec_in = nc.dram_tensor("dect", [NKD, 128, BPC], F16, kind="ExternalInput")
    out_dram = nc.dram_tensor("out", [BPC, 128, NFT], F32, kind="ExternalOutput")
    if PE_WS_LAST:
        sf_in = nc.dram_tensor("enc_sf", [S // 128, 128, F], F16,
                               kind="ExternalInput")
        out2_dram = nc.dram_tensor("out2", [1, F], F32, kind="ExternalOutput")

    Tanh = mybir.ActivationFunctionType.Tanh
    Exp = mybir.ActivationFunctionType.Exp
    AX = mybir.AxisListType.X
    MUL = mybir.AluOpType.mult
    ADD = mybir.AluOpType.add

    h_scale = (1.0 / W_SCALE) if fp8 else 1.0

    if EBC == "pool":
        from concourse import library_config
        nc.gpsimd.load_library(library_config.attn)

    with tile.TileContext(nc) as tc:
        with (
            tc.tile_pool(name="consts", bufs=1) as consts,
            tc.tile_pool(name="encp", bufs=3 * NFT) as encp,
            tc.tile_pool(name="enc8p", bufs=3 * NQ) as enc8p,
            tc.tile_pool(name="thp", bufs=2 * NM + 2) as thp,
            tc.tile_pool(name="ep", bufs=3) as ep_pool,
            tc.tile_pool(name="zp", bufs=3) as zp,
            tc.tile_pool(name="scrp", bufs=4) as scrp,
            tc.tile_pool(name="sfp", bufs=2) as sfp,
            tc.tile_pool(name="etp", bufs=2) as etp,
            tc.tile_pool(name="accp", bufs=3) as accp,
            tc.tile_pool(name="outp", bufs=3) as outp_pool,
            # 5 hT banks + 1 score bank + 2 misc = 8: the extra hT bank lets
            # the next block's first m-group start while the previous block's
            # last tanh still reads its bank; the single score bank drains
            # fast via the exp read.
            tc.tile_pool(name="hps", bufs=5, space="PSUM") as hps,
            tc.tile_pool(name="scps", bufs=1, space="PSUM") as scps,
            tc.tile_pool(name="miscps", bufs=2, space="PSUM") as miscps,
        ):
            # ---- constants (small dec-path operands first: PE work sooner) ----
            w1d_sb = consts.tile([128, NKD, D], F16)
            nc.sync.dma_start(out=w1d_sb, in_=w1d_in.rearrange("a p d -> p a d"))
            b1_sb = consts.tile([1, D], F16)
            nc.sync.dma_start(out=b1_sb, in_=b1_in[:, :])
            if use_wcorr:
                w2_sb = consts.tile([128, NM, 2], F16)
                nc.sync.dma_start(out=w2_sb,
                                  in_=w2_in.rearrange("a p o -> p a o"))
            else:
                w2_sb = consts.tile([128, NM], F16)
                nc.sync.dma_start(out=w2_sb, in_=w2_in.rearrange("a p -> p a"))
            dect_sb = consts.tile([128, NKD, BPC], F16)
            nc.sync.dma_start(out=dect_sb, in_=dec_in.rearrange("a p b -> p a b"))

            enc0_8 = []
            enc0_tiles = []
            if fp8:
                # interleave weight chunks with batch-0 fp8 enc tiles so the
                # PE can start the kt-outer first block while streaming
                w1e8h_sb = consts.tile([128, NQ, 2, D], F8)
                if use_wlo:
                    w1e8l_sb = consts.tile([128, NQ, 2, D], F8)
                if use_wcorr:
                    c8_sb = consts.tile([128, NQ, 2, 128], F8)
                    nc.sync.dma_start(
                        out=c8_sb, in_=c8_in.rearrange("a p t o -> p a t o")
                    )
                for q in range(NQ):
                    nc.sync.dma_start(out=w1e8h_sb[:, q], in_=w1e8h_in[q])
                    if use_wlo:
                        nc.sync.dma_start(out=w1e8l_sb[:, q], in_=w1e8l_in[q])
                    e8t = enc8p.tile([128, 2, S], F8, tag="enc8_t", name="enc8_t")
                    nc.sync.dma_start(out=e8t, in_=enc8_in[0, q])
                    enc0_8.append(e8t)
                for ft in range(NFT):
                    et0 = encp.tile([128, S], F16, tag="enc_t", name="enc_t")
                    nc.sync.dma_start(out=et0, in_=enc_in[0, ft])
                    enc0_tiles.append(et0)
            else:
                w1e_sb = consts.tile([128, NFT, D], F16)
                for kt in range(NFT):
                    nc.sync.dma_start(out=w1e_sb[:, kt, :], in_=w1e_in[kt])
                    et0 = encp.tile([128, S], F16, tag="enc_t", name="enc_t")
                    nc.sync.dma_start(out=et0, in_=enc_in[0, kt])
                    enc0_tiles.append(et0)

            onesb = consts.tile([1, BPC], F16)
            nc.vector.memset(onesb, 1.0)
            ones16 = consts.tile([1, 128], F16)
            nc.vector.memset(ones16, 1.0)
            ones32 = consts.tile([1, 128], F32)
            nc.vector.memset(ones32, 1.0)
            shift_sb = consts.tile([1, 1], F32)
            nc.vector.memset(shift_sb, EXP_SHIFT)
            if PE_WS_LAST:
                shift128 = consts.tile([128, 1], F32)
                nc.vector.memset(shift128, EXP_SHIFT)

            # ---- dec_proj^T + b1:  [128 (d%), NM * BPC] fp32 ----
            dec_sb = consts.tile([128, NM, BPC], F32)
            for m in range(NM):
                dp = miscps.tile([128, BPC], F32, tag="mps")
                for kd in range(NKD):
                    nc.tensor.matmul(
                        dp,
                        lhsT=w1d_sb[:, kd, m * 128:(m + 1) * 128],
                        rhs=dect_sb[:, kd, :],
                        start=(kd == 0),
                        stop=False,
                    )
                nc.tensor.matmul(
                    dp,
                    lhsT=b1_sb[0:1, m * 128:(m + 1) * 128],
                    rhs=onesb,
                    start=False,
                    stop=True,
                )
                nc.scalar.copy(dec_sb[:, m, :], dp)

            def h_matmuls(hp, m, e8_tiles, e16_tiles, sl, q_outer_bufs=None):
                """Accumulate one m-tile of hT over the full F contraction."""
                if fp8:
                    w_list = [w1e8h_sb] + ([w1e8l_sb] if use_wlo else [])
                    if HILO_ORDER == "q_major":
                        seq = [(w_sb, q) for q in range(NQ) for w_sb in w_list]
                    else:
                        seq = [(w_sb, q) for w_sb in w_list for q in range(NQ)]
                    for j, (w_sb, q) in enumerate(seq):
                        nc.tensor.matmul(
                            hp,
                            lhsT=w_sb[:, q, :, m * 128:(m + 1) * 128],
                            rhs=e8_tiles[q][:, :, sl],
                            start=(j == 0),
                            stop=(j == len(seq) - 1),
                            perf_mode=DR,
                        )
                else:
                    for kt in range(NFT):
                        nc.tensor.matmul(
                            hp,
                            lhsT=w1e_sb[:, kt, m * 128:(m + 1) * 128],
                            rhs=e16_tiles[kt][:, sl],
                            start=(kt == 0),
                            stop=(kt == NFT - 1),
                        )

            # ---- main loop over local batches ----
            for iter_i, b in enumerate(i % BPC for i in range(repeat * BPC)):
                if iter_i == 0:
                    enc_tiles = enc0_tiles
                    enc8_tiles = enc0_8
                else:
                    enc8_tiles = []
                    if fp8:
                        for q in range(NQ):
                            e8t = enc8p.tile([128, 2, S], F8, tag="enc8_t")
                            nc.sync.dma_start(out=e8t, in_=enc8_in[b, q])
                            enc8_tiles.append(e8t)
                    enc_tiles = []
                    for ft in range(NFT):
                        et = encp.tile([128, S], F16, tag="enc_t")
                        nc.sync.dma_start(out=et, in_=enc_in[b, ft])
                        enc_tiles.append(et)

                blocks = [(i * SBLK, SBLK) for i in range(NSB)]

                pe_ws = PE_WS_LAST and b == BPC - 1
                if pe_ws:
                    sf_sb = sfp.tile([128, S // 128, F], F16, tag="sf")
                    nc.sync.dma_start(
                        out=sf_sb, in_=sf_in.rearrange("a p f -> p a f")
                    )
                    eT_sb = etp.tile([128, S // 128], F16, tag="eT")

                e16 = ep_pool.tile([1, S], F16)
                zacc = zp.tile([1, len(blocks)], F32, tag="zacc")
                acc2 = accp.tile([128, len(blocks), NFT], F32)

                first_block = iter_i == 0
                for sb, (boff, bsz) in enumerate(blocks):
                    sl = slice(boff, boff + bsz)
                    th_tiles = []
                    if first_block and sb == 0 and fp8:
                        # q-outer: every arriving enc8 tile feeds its matmuls
                        hp_list = [hps.tile([128, bsz], F32, tag="hp", name="hp")
                                   for _ in range(NM)]
                        passes = [(w1e8h_sb, True)] + (
                            [(w1e8l_sb, False)] if use_wlo else []
                        )
                        last_w = passes[-1][0]
                        for q in range(NQ):
                            for w_sb, is_first in passes:
                                for m in range(NM):
                                    nc.tensor.matmul(
                                        hp_list[m],
                                        lhsT=w_sb[:, q, :, m * 128:(m + 1) * 128],
                                        rhs=enc8_tiles[q][:, :, sl],
                                        start=(is_first and q == 0),
                                        stop=(w_sb is last_w and q == NQ - 1),
                                        perf_mode=DR,
                                    )
                        for m in range(NM):
                            th = thp.tile([128, bsz], F16, tag="th")
                            nc.scalar.activation(
                                th, hp_list[m], Tanh,
                                bias=dec_sb[:, m, b:b + 1], scale=h_scale,
                            )
                            th_tiles.append(th)
                    elif first_block and sb == 0:
                        hp_list = [hps.tile([128, bsz], F32, tag="hp", name="hp")
                                   for _ in range(NM)]
                        for kt in range(NFT):
                            for m in range(NM):
                                nc.tensor.matmul(
                                    hp_list[m],
                                    lhsT=w1e_sb[:, kt, m * 128:(m + 1) * 128],
                                    rhs=enc_tiles[kt][:, sl],
                                    start=(kt == 0),
                                    stop=(kt == NFT - 1),
                                )
                        for m in range(NM):
                            th = thp.tile([128, bsz], F16, tag="th")
                            nc.scalar.activation(
                                th, hp_list[m], Tanh,
                                bias=dec_sb[:, m, b:b + 1], scale=h_scale,
                            )
                            th_tiles.append(th)
                    else:
                        for m in range(NM):
                            hp = hps.tile([128, bsz], F32, tag="hp")
                            h_matmuls(hp, m, enc8_tiles, enc_tiles, sl)
                            th = thp.tile([128, bsz], F16, tag="th")
                            nc.scalar.activation(
                                th, hp, Tanh, bias=dec_sb[:, m, b:b + 1],
                                scale=h_scale,
                            )
                            th_tiles.append(th)

                    if use_wcorr:
                        # rank-1 W-residual correction rides the score psum
                        # (runs early: only needs enc8, not th). M=128 because
                        # dual-fp8 ldweights requires it; rows 1..127 junk.
                        sc2 = scps.tile([128, bsz], F32, tag="sc")
                        for q in range(NQ):
                            nc.tensor.matmul(
                                sc2,
                                lhsT=c8_sb[:, q, :, :],
                                rhs=enc8_tiles[q][:, :, sl],
                                start=(q == 0),
                                stop=False,
                                perf_mode=DR,
                            )
                        sc = sc2[0:1, :]
                        for m in range(NM):
                            nc.tensor.matmul(
                                sc,
                                lhsT=w2_sb[:, m, 0:1],
                                rhs=th_tiles[m],
                                start=False,
                                stop=(m == NM - 1),
                            )
                    else:
                        sc = scps.tile([1, bsz], F32, tag="sc")
                        for m in range(NM):
                            nc.tensor.matmul(
                                sc,
                                lhsT=w2_sb[:, m:m + 1],
                                rhs=th_tiles[m],
                                start=(m == 0),
                                stop=(m == NM - 1),
                            )
                    # e = exp(scores * sc_scale + EXP_SHIFT); zacc[sb] = sum(e)
                    nc.scalar.activation(
                        e16[0:1, sl], sc, Exp,
                        bias=shift_sb[0:1, 0:1],
                        scale=sc_scale,
                        accum_out=zacc[0:1, sb:sb + 1],
                    )
                    if pe_ws:
                        for c in range(bsz // 128):
                            scT = miscps.tile([128, 1], F32, tag="mps",
                                              name="scT")
                            for kd in range(NM):
                                nc.tensor.matmul(
                                    scT,
                                    lhsT=th_tiles[kd][:, c * 128:(c + 1) * 128],
                                    rhs=w2_sb[:, kd:kd + 1],
                                    start=(kd == 0),
                                    stop=(kd == NM - 1),
                                )
                            ci = boff // 128 + c
                            nc.scalar.activation(
                                eT_sb[:, ci:ci + 1], scT, Exp,
                                bias=shift128[:, 0:1],
                            )
                        continue
                    if EBC == "pool":
                        ebc16 = scrp.tile([128, bsz], F16, tag="ebc16")
                        nc.gpsimd.partition_broadcast(ebc16, e16[0:1, sl])
                        ebc = ebc16
                    else:
                        ebc = miscps.tile([128, bsz], F32, tag="mps")
                        nc.tensor.matmul(
                            ebc, lhsT=ones16, rhs=e16[0:1, sl],
                            start=True, stop=True,
                        )
                        if STT16:
                            ebc16 = scrp.tile([128, bsz], F16, tag="ebc16")
                            nc.vector.tensor_copy(ebc16, ebc)
                            ebc = ebc16
                    for ft in range(NFT):
                        scr = scrp.tile([128, bsz], F16, tag="scr")
                        nc.vector.scalar_tensor_tensor(
                            out=scr,
                            in0=enc_tiles[ft][:, sl],
                            scalar=1.0,
                            in1=ebc,
                            op0=MUL,
                            op1=MUL,
                            accum_out=acc2[:, sb, ft:ft + 1],
                        )

                z = zp.tile([1, 1], F32, tag="z")
                nc.vector.tensor_reduce(z, zacc, axis=AX, op=ADD)
                invz = zp.tile([1, 1], F32, tag="invz")
                nc.vector.reciprocal(invz, z)

                if pe_ws:
                    o1 = outp_pool.tile([1, F], F32, tag="o1")
                    for fo in range(0, F, 512):
                        w = min(512, F - fo)
                        aps = hps.tile([1, w], F32, tag="hp", name="aps")
                        for kt in range(S // 128):
                            nc.tensor.matmul(
                                aps,
                                lhsT=eT_sb[:, kt:kt + 1],
                                rhs=sf_sb[:, kt, fo:fo + w],
                                start=(kt == 0),
                                stop=(kt == S // 128 - 1),
                            )
                        nc.scalar.activation(
                            o1[0:1, fo:fo + w], aps,
                            mybir.ActivationFunctionType.Copy,
                            scale=invz[0:1, 0:1],
                        )
                    nc.sync.dma_start(out=out2_dram[0:1, :], in_=o1)
                    continue

                izp = scps.tile([128, 1], F32, tag="sc")
                nc.tensor.matmul(izp, lhsT=ones32, rhs=invz, start=True, stop=True)
                izb = zp.tile([128, 1], F32, tag="izb")
                nc.scalar.copy(izb, izp)

                o = outp_pool.tile([128, NFT], F32)
                nblk = len(blocks)
                if nblk == 1:
                    nc.vector.tensor_scalar_mul(o, acc2[:, 0, :], izb)
                else:
                    osum = outp_pool.tile([128, NFT], F32, tag="osum")
                    nc.vector.tensor_add(osum, acc2[:, 0, :], acc2[:, 1, :])
                    for sb in range(2, nblk):
                        nc.vector.tensor_add(osum, osum, acc2[:, sb, :])
                    nc.vector.tensor_scalar_mul(o, osum, izb)
                nc.sync.dma_start(out=out_dram[b], in_=o)

    _split_multi_waits(nc)
    return nc


def prep_inputs(encoder_output, decoder_hidden, W1, b1, W2, cfg: Cfg):
    """Host-side sharding + layout. Returns per-core input maps."""
    S, F, D, DEC, BPC = cfg.S, cfg.F, cfg.D, cfg.DEC, cfg.BPC
    NFT, NQ, NM, NKD = cfg.NFT, cfg.NQ, cfg.NM, cfg.NKD
    n_cores = encoder_output.shape[1] // BPC
    fp8 = QUANT in ("fp8_pure", "fp8_wlo", "fp8_wcorr")

    # enc [S, B, F] -> [B, F, S] -> f = p*NFT + ft -> [B, NFT, 128, S] fp16
    enc_t = np.ascontiguousarray(encoder_output.transpose(1, 2, 0))
    enc_l = np.ascontiguousarray(
        enc_t.reshape(-1, 128, NFT, S).transpose(0, 2, 1, 3)
    ).astype(np.float16)

    w1e_t = W1[:, :F].T.astype(np.float32)  # [F, D]
    if fp8:
        # f = q*256 + i*128 + p  (DoubleRow pair layout)
        enc8_l = np.ascontiguousarray(
            enc_t.reshape(-1, NQ, 2, 128, S).transpose(0, 1, 3, 2, 4)
        ).astype(NPF8)
        w16 = w1e_t * W_SCALE
        whi = w16.astype(NPF8)
        wlo = (w16 - whi.astype(np.float32)).astype(NPF8)
        w1e8h = np.ascontiguousarray(
            whi.reshape(NQ, 2, 128, D).transpose(0, 2, 1, 3)
        )
        w1e8l = np.ascontiguousarray(
            wlo.reshape(NQ, 2, 128, D).transpose(0, 2, 1, 3)
        )
        if QUANT == "fp8_wcorr":
            dW = w1e_t - whi.astype(np.float32) / W_SCALE  # [F, D]
            c = dW @ (W2.reshape(-1) * C0)                 # [F]
            c8 = (c * C_SCALE).astype(NPF8)
            c8_l = np.zeros((NQ, 128, 2, 128), dtype=NPF8)
            c8_l[..., 0] = c8.reshape(NQ, 2, 128).transpose(0, 2, 1)
    else:
        w1e = np.ascontiguousarray(
            w1e_t.reshape(128, NFT, D).transpose(1, 0, 2)
        ).astype(np.float16)

    w1d_t = W1[:, F:].T.astype(np.float32)  # [DEC, D]
    w1d = np.ascontiguousarray(w1d_t.reshape(NKD, 128, D)).astype(np.float16)

    b1r = b1.reshape(1, D).astype(np.float16)
    if QUANT == "fp8_wcorr":
        w2t = np.zeros((NM, 128, 2), dtype=np.float16)
        w2t[..., 0] = (W2.reshape(D) * C_SCALE).reshape(NM, 128)
    else:
        w2t = W2.reshape(D).reshape(NM, 128).astype(np.float16)
    dect_full = decoder_hidden.T.reshape(NKD, 128, -1).astype(np.float16)

    in_maps = []
    for c in range(n_cores):
        bs = slice(c * BPC, (c + 1) * BPC)
        m = {
            "enc_in": enc_l[bs],
            "w1d": w1d,
            "b1r": b1r,
            "w2t": w2t,
            "dect": np.ascontiguousarray(dect_full[:, :, bs]),
        }
        if fp8:
            m["enc8"] = enc8_l[bs]
            m["w1e8h"] = w1e8h
            if QUANT == "fp8_wlo":
                m["w1e8l"] = w1e8l
            if QUANT == "fp8_wcorr":
                m["c8"] = c8_l
        else:
            m["w1e"] = w1e
        if PE_WS_LAST:
            sf = np.ascontiguousarray(
                encoder_output[:, c * BPC + BPC - 1, :]
            ).astype(np.float16)
            m["enc_sf"] = sf.reshape(S // 128, 128, F)
        in_maps.append(m)
    return in_maps


def assemble_core(r, cfg: Cfg):
    """Per-core output rows [BPC, F] from the result map."""
    out = r["out"].reshape(cfg.BPC, cfg.F).copy()
    if PE_WS_LAST:
        out[cfg.BPC - 1] = r["out2"][0]
    return out


def kernel(encoder_output, decoder_hidden, W1, b1, W2, b2):
    """Full inputs in, full output out. b2 cancels in the softmax."""
    encoder_output = np.asarray(encoder_output, dtype=np.float32)
    decoder_hidden = np.asarray(decoder_hidden, dtype=np.float32)
    W1 = np.asarray(W1, dtype=np.float32)
    b1 = np.asarray(b1, dtype=np.float32)
    W2 = np.asarray(W2, dtype=np.float32)

    cfg = FULL
    nc = build_bass(cfg)
    in_maps = prep_inputs(encoder_output, decoder_hidden, W1, b1, W2, cfg)
    res = run_bass_kernel_spmd(nc, in_maps, list(range(N_CORES)))
    out = np.concatenate(
        [assemble_core(r, cfg) for r in res.results], axis=0
    )
    return out[:, None, :].astype(np.float32)


if __name__ == "__main__":
    import reference

    inputs = reference.setup_inputs()
    expected = np.asarray(reference.reference(**inputs))
    actual = kernel(**{k: np.asarray(v) for k, v in inputs.items()})
    err = np.abs(actual - expected).max() / np.abs(expected).max()
    print("Relative error:", err)
